# revision 1
# baseline (speedup 1.0000x reference)
"""Multi-head attention (B=2, S=2048, D=1024, H=16) on 8 TRN2 NeuronCores.

Sharding: data-parallel over batch (2) x tensor-parallel over head groups (4).
Core c handles batch c//4, heads 4*(c%4) .. 4*(c%4)+3 (256 projection dims).
Each core computes its partial output projection; the host sums the 4 partials
per batch and adds the (bv @ wo.T + bo) constant, which is exact because
softmax weights sum to 1.

Device layouts (per core):
  QT/KT  [128, 2, 2048] bf16 : partition p + 128*m = local proj dim, free = seq
  V_sb   [128, 16, 4, 65] bf16: [k-pos within tile, k-tile, head, dk + ones col]
  scores computed transposed: ST[k, q] = K'h @ Q'h^T, exp on ScalarE,
  PV: attnT[dk, q] += [Vh | 1]^T @ expST  (ones column yields softmax denom)
  out-proj: out[s, dout] = attnT^T @ woT, partial, f32 to DRAM.
"""

import sys

sys.path.insert(0, "/opt/trn_rl_repo")

import numpy as np
import ml_dtypes

BF16 = ml_dtypes.bfloat16

B, S, D = 2, 2048, 1024
H, DK = 16, 64
N_CORES = 8
GROUPS = 4  # head groups (tensor-parallel)
DL = D // GROUPS  # 256 local projection dims per core
SCALE = 1.0 / np.sqrt(np.sqrt(float(DK)))  # fold 1/sqrt(dk) half into Q, half into K

_cache: dict = {}


def _build():
    import concourse.mybir as mybir
    import concourse.tile as tile
    from concourse import bacc

    dt = mybir.dt
    f32, bf16 = dt.float32, dt.bfloat16

    nc = bacc.Bacc("TRN2", target_bir_lowering=False, debug=False,
                   num_devices=N_CORES)

    xqT = nc.dram_tensor("xqT", [D, S], bf16, kind="ExternalInput").ap()
    xkT = nc.dram_tensor("xkT", [D, S], bf16, kind="ExternalInput").ap()
    xvT = nc.dram_tensor("xvT", [D, S], bf16, kind="ExternalInput").ap()
    wqT = nc.dram_tensor("wqT", [D, DL], bf16, kind="ExternalInput").ap()
    wkT = nc.dram_tensor("wkT", [D, DL], bf16, kind="ExternalInput").ap()
    wvT = nc.dram_tensor("wvT", [D, DL], bf16, kind="ExternalInput").ap()
    woT = nc.dram_tensor("woT", [DL, D], bf16, kind="ExternalInput").ap()
    bqk = nc.dram_tensor("bqk", [2, DL], f32, kind="ExternalInput").ap()
    out = nc.dram_tensor("out", [S, D], f32, kind="ExternalOutput").ap()

    EXPF = mybir.ActivationFunctionType.Exp

    with tile.TileContext(nc) as tc:
        with (
            tc.tile_pool(name="res", bufs=1) as res,
            tc.tile_pool(name="wts", bufs=1) as wts,
            tc.tile_pool(name="xin", bufs=3) as xin,
            tc.tile_pool(name="expp", bufs=6) as expp,
            tc.tile_pool(name="nrm", bufs=3) as nrm,
            tc.tile_pool(name="drm", bufs=2, space="DRAM") as drm,
            tc.tile_pool(name="ps_proj", bufs=2, space="PSUM") as ps_proj,
            tc.tile_pool(name="ps_at", bufs=2, space="PSUM") as ps_at,
            tc.tile_pool(name="ps_st", bufs=2, space="PSUM") as ps_st,
        ):
            # ---- resident tensors ----
            QT = [res.tile([128, S], bf16, name=f"QT{m}", tag=f"QT{m}")
                  for m in range(2)]
            KT = [res.tile([128, S], bf16, name=f"KT{m}", tag=f"KT{m}")
                  for m in range(2)]
            Vsb = res.tile([128, 16, 4, DK + 1], bf16)
            ATT = [res.tile([128, S], bf16, name=f"ATT{m}", tag=f"ATT{m}")
                   for m in range(2)]

            wq_sb = wts.tile([128, 8, DL], bf16, tag="wq")
            wk_sb = wts.tile([128, 8, DL], bf16, tag="wk")
            wv_sb = wts.tile([128, 8, DL], bf16, tag="wv")
            wo_sb = wts.tile([128, 2, D], bf16, tag="wo")
            b_sb = wts.tile([128, 2, 2], f32, tag="b")  # [p, proj(q/k), m]

            nc.vector.memset(Vsb[:, :, :, DK], 1.0)

            def qk_proj_fillers(m, pj, sb):
                """Two small closures: DMA + first half of the k-accumulation,
                then second half + eviction. Dripped into attention's kt loop
                so they never hog PE long enough to starve ScalarE."""
                xsrc, wsb, dst = [(xqT, wq_sb, QT), (xkT, wk_sb, KT)][pj]
                state = {}

                def part1():
                    xb = xin.tile([128, 8, 512], bf16, tag="xblk",
                                  name=f"xb{m}{pj}{sb}")
                    nc.gpsimd.dma_start(
                        xb[:, :, :],
                        xsrc[:, sb * 512:(sb + 1) * 512].rearrange(
                            "(n p) d -> p n d", p=128))
                    ps = ps_proj.tile([128, 512], f32, tag="proj",
                                      name=f"psp{m}{pj}{sb}")
                    for kt in range(4):
                        nc.tensor.matmul(
                            ps[:, :],
                            lhsT=wsb[:, kt, m * 128:(m + 1) * 128],
                            rhs=xb[:, kt, :],
                            start=(kt == 0), stop=False)
                    state["xb"], state["ps"] = xb, ps

                def part2():
                    xb, ps = state["xb"], state["ps"]
                    for kt in range(4, 8):
                        nc.tensor.matmul(
                            ps[:, :],
                            lhsT=wsb[:, kt, m * 128:(m + 1) * 128],
                            rhs=xb[:, kt, :],
                            start=False, stop=(kt == 7))
                    nc.vector.tensor_scalar_add(
                        dst[m][:, sb * 512:(sb + 1) * 512],
                        ps[:, :], b_sb[:, pj, m:m + 1])

                return [part1, part2]

            def emit_qk_proj(m, pj, sbs):
                for sb in sbs:
                    for f in qk_proj_fillers(m, pj, sb):
                        f()

            def emit_v_stq(stq):
                xb = xin.tile([128, 8, 512], bf16, tag="xblk",
                              name=f"xbv{stq}")
                nc.gpsimd.dma_start(
                    xb[:, :, :],
                    xvT[:, stq * 512:(stq + 1) * 512].rearrange(
                        "(n p) d -> p n d", p=128))
                for sts in range(4):
                    st = stq * 4 + sts
                    ps = ps_proj.tile([128, 4, DK], f32, tag="proj",
                                      name=f"psv{st}")
                    for kt in range(8):
                        nc.tensor.matmul(
                            ps[:, :, :],
                            lhsT=xb[:, kt, sts * 128:(sts + 1) * 128],
                            rhs=wv_sb[:, kt, :],
                            start=(kt == 0), stop=(kt == 7))
                    nc.vector.tensor_copy(Vsb[:, st, :, 0:DK], ps[:, :, :])

            def alloc_pa(hp, qb):
                return [ps_at.tile([65, 512], f32, tag="at",
                                   name=f"at{hp}{qb}{i}") for i in range(2)]

            def emit_attn_kts(hp, qb, pa, kt_range, fillers=None):
                qs = slice(qb * 512, (qb + 1) * 512)
                for kt in kt_range:
                    # both heads' score tiles back-to-back: disjoint row
                    # groups (partitions 0-63 / 64-127) co-stream on PE
                    st_ps = ps_st.tile([128, 2, 512], f32, tag="st")
                    for hh in range(2):
                        lo, hi = hh * 64, hh * 64 + 64
                        nc.tensor.matmul(
                            st_ps[:, hh, :],
                            lhsT=KT[hp][lo:hi, kt * 128:(kt + 1) * 128],
                            rhs=QT[hp][lo:hi, qs],
                            start=True, stop=True)
                    ex = expp.tile([128, 2, 512], bf16, tag="exp")
                    nc.scalar.activation(ex[:, :, :], st_ps[:, :, :], EXPF)
                    for hh in range(2):
                        nc.tensor.matmul(
                            pa[hh][:, :],
                            lhsT=Vsb[:, kt, 2 * hp + hh, :],
                            rhs=ex[:, hh, :],
                            start=(kt == 0), stop=(kt == 15),
                            skip_group_check=True)
                    if fillers:
                        fillers.popleft()()

            def emit_attn_norm(hp, qb, pa):
                # normalize rows 0..63 by row 64, write into ATT
                qs = slice(qb * 512, (qb + 1) * 512)
                for hh in range(2):
                    asb = nrm.tile([65, 512], f32, tag="asb")
                    nc.vector.tensor_copy(asb[:, :], pa[hh][:, :])
                    rdr = drm.tile([1, 512], f32, tag="rdr")
                    nc.sync.dma_start(rdr[:, :], asb[64:65, :])
                    rq = nrm.tile([128, 4], f32, tag="rq")
                    nc.sync.dma_start(
                        rq[:, :], rdr[0, :].rearrange("(p f) -> p f", p=128))
                    rq2 = nrm.tile([128, 4], f32, tag="rq2")
                    nc.vector.reciprocal(rq2[:, :], rq[:, :])
                    rdr2 = drm.tile([1, 512], f32, tag="rdr2")
                    nc.sync.dma_start(
                        rdr2[0, :].rearrange("(p f) -> p f", p=128), rq2[:, :])
                    rb = nrm.tile([64, 512], f32, tag="rb")
                    nc.sync.dma_start(rb[:, :],
                                      rdr2[:, :].to_broadcast((64, 512)))
                    if hh == 0:
                        nc.vector.tensor_mul(ATT[hp][0:64, qs],
                                             asb[0:64, :], rb[:, :])
                    else:
                        tmp = nrm.tile([64, 512], bf16, tag="tmp")
                        nc.vector.tensor_mul(tmp[:, :], asb[0:64, :], rb[:, :])
                        nc.sync.dma_start(ATT[hp][64:128, qs], tmp[:, :])

            def emit_attn_qb(hp, qb):
                pa = alloc_pa(hp, qb)
                emit_attn_kts(hp, qb, pa, range(16))
                emit_attn_norm(hp, qb, pa)

            def out_proj_filler(st, db):
                def f():
                    ps = ps_proj.tile([128, 512], f32, tag="proj",
                                      name=f"pso{st}{db}")
                    for m in range(2):
                        nc.tensor.matmul(
                            ps[:, :],
                            lhsT=ATT[m][:, st * 128:(st + 1) * 128],
                            rhs=wo_sb[:, m, db * 512:(db + 1) * 512],
                            start=(m == 0), stop=(m == 1))
                    osb = nrm.tile([128, 512], f32, tag="osb")
                    nc.vector.tensor_copy(osb[:, :], ps[:, :])
                    nc.gpsimd.dma_start(
                        out[st * 128:(st + 1) * 128, db * 512:(db + 1) * 512],
                        osb[:, :])
                return f

            def emit_out_proj(sts):
                for st in sts:
                    for db in range(2):
                        out_proj_filler(st, db)()

            # ---- emission: flash-style streaming. K/V/Q chunks feed
            # attention's k-tile pipeline incrementally; A1 and the output
            # projection gap-fill PE while ScalarE (exp) runs flat out ----
            for kt in range(8):
                nc.sync.dma_start(wk_sb[:, kt, :],
                                    wkT[kt * 128:(kt + 1) * 128, :])
                nc.sync.dma_start(wv_sb[:, kt, :],
                                    wvT[kt * 128:(kt + 1) * 128, :])
            for m in range(2):
                for pj in range(2):
                    nc.sync.dma_start(b_sb[:, pj, m:m + 1],
                                        bqk[pj, m * 128:(m + 1) * 128, None])
            for kt in range(8):
                nc.sync.dma_start(wq_sb[:, kt, :],
                                    wqT[kt * 128:(kt + 1) * 128, :])

            # qb0 of C0 streams against its producers: each K/V seq-block
            # lands just before the k-tile pairs that consume it
            emit_qk_proj(0, 1, [0])
            emit_v_stq(0)
            emit_qk_proj(0, 0, [0])
            pa0 = alloc_pa(0, 0)
            for blk in range(4):
                emit_attn_kts(0, 0, pa0, range(4 * blk, 4 * blk + 4))
                if blk < 3:
                    emit_qk_proj(0, 1, [blk + 1])
                    emit_v_stq(blk + 1)
            emit_attn_norm(0, 0, pa0)
            for m in range(2):
                nc.sync.dma_start(wo_sb[:, m, :],
                                  woT[m * 128:(m + 1) * 128, :])

            from collections import deque

            # A1 (m=1 projections) dripped into C0's qb1-3 kt loops
            a1 = deque()
            for pj in range(2):
                for sb in range(4):
                    a1.extend(qk_proj_fillers(1, pj, sb))
            for qb in range(1, 4):
                emit_qk_proj(0, 0, [qb])
                pa = alloc_pa(0, qb)
                emit_attn_kts(0, qb, pa, range(16), fillers=a1)
                emit_attn_norm(0, qb, pa)
            while a1:
                a1.popleft()()

            # out-projection dripped into C1's qb1-3 kt loops, one q-block
            # of ATT behind the attention that produces it
            dq = deque()
            for qb in range(4):
                pa = alloc_pa(1, qb)
                emit_attn_kts(1, qb, pa, range(16), fillers=dq)
                emit_attn_norm(1, qb, pa)
                for st in range(qb * 4, qb * 4 + 4):
                    for db in range(2):
                        dq.append(out_proj_filler(st, db))
            while dq:
                dq.popleft()()

    nc.compile()
    return nc


def _prep_inputs(q, k, v, wq, bq, wk, bk, wv, bv, wo, bo):
    q, k, v = (np.asarray(a, np.float32) for a in (q, k, v))
    wq, bq, wk, bk, wv, bv, wo, bo = (
        np.asarray(a, np.float32) for a in (wq, bq, wk, bk, wv, bv, wo, bo))

    xT = {}
    for b in range(B):
        xT[("q", b)] = np.ascontiguousarray(q[b].T).astype(BF16)
        xT[("k", b)] = np.ascontiguousarray(k[b].T).astype(BF16)
        xT[("v", b)] = np.ascontiguousarray(v[b].T).astype(BF16)

    grp = {}
    for g in range(GROUPS):
        hs = slice(g * DL, (g + 1) * DL)
        grp[g] = {
            "wqT": np.ascontiguousarray((wq[hs, :] * SCALE).T).astype(BF16),
            "wkT": np.ascontiguousarray((wk[hs, :] * SCALE).T).astype(BF16),
            "wvT": np.ascontiguousarray(wv[hs, :].T).astype(BF16),
            "woT": np.ascontiguousarray(wo[:, hs].T).astype(BF16),
            "bqk": np.stack([bq[hs] * SCALE, bk[hs] * SCALE]).astype(np.float32),
        }

    in_maps = []
    for c in range(N_CORES):
        b, g = c // GROUPS, c % GROUPS
        m = {"xqT": xT[("q", b)], "xkT": xT[("k", b)], "xvT": xT[("v", b)]}
        m.update(grp[g])
        in_maps.append(m)

    const = (bv @ wo.T + bo).astype(np.float32)  # exact since sum(P) == 1
    return in_maps, const


def _run(in_maps, trace=False):
    from concourse.bass_utils import run_bass_kernel_spmd

    if "nc" not in _cache:
        _cache["nc"] = _build()
    return run_bass_kernel_spmd(_cache["nc"], in_maps, list(range(N_CORES)),
                                trace=trace)


def _reduce(results, const):
    out = np.zeros((B, S, D), np.float32)
    for c in range(N_CORES):
        out[c // GROUPS] += results[c]["out"]
    out += const
    return out


def kernel(**inputs) -> np.ndarray:
    in_maps, const = _prep_inputs(**inputs)
    res = _run(in_maps, trace=False)
    return _reduce(res.results, const)


def kernel_profiled(**inputs):
    """Returns (output, exec_time_ns or None)."""
    in_maps, const = _prep_inputs(**inputs)
    res = _run(in_maps, trace=True)
    return _reduce(res.results, const), res.exec_time_ns

